# revision 4
# baseline (speedup 1.0000x reference)
"""DCT2net denoiser on 8 TRN2 NeuronCores.

Sharding: 8 cores = 4 images x 2 horizontal half-bands (data-parallel,
halo via overlapping patch bands -- no collectives).

Math: with forward weights Wf = Pm1/lam (threshold folded to +-1) and
z = t * 1{|t|<=1} (the sub-threshold coefficients), the hardshrink
reconstruction satisfies exactly

    rec = patches - lam*Pinv @ z

and since fold(w * shifted-copies-of-image) == image * fold(w), the
final output reduces to

    out = x - fold(w * recm)/fold(w) / 2,    recm := lam*Pinv @ z

so the device only computes t (fp32r matmul over on-device im2col),
z, cnt = #sub-threshold AC coeffs, and recm (f16 matmul).  Host does
reflect-pad, the overlap-add fold and the final division.

Device per core: im2col via overlapping-stride DMA from the small DRAM
image (no host im2col, no big input transfer), forward transform
(fp32r, optional residual pass), Square+is_le -> indbar, z = t*indbar,
count via ones-matmul packed into a spare PSUM partition, inverse
transform (f16), ship recm f16 + cnt f32.
"""

import numpy as np

P = 13
PP = 169              # p*p
N_IMG, H, W = 4, 256, 256
BAND_OUT = 128        # output rows per core
PATCH_ROWS = BAND_OUT + P - 1        # 140
BAND_ROWS = BAND_OUT + 2 * (P - 1)   # 152
WPAD = W + 2 * (P - 1)               # 280
WO = W + P - 1        # 268 patch cols
L = PATCH_ROWS * WO   # 37520
KA, KB = 117, 52      # patch-row split: di groups 0..8 / 9..12
MA, MB = 128, 41      # coefficient split (k index)
NT = 512              # free-dim tile
HB = 21               # patch rows per staged chunk (21*268 = 5628 = 10*512+508)

# forward-transform mode: "f32r" (single high pass), "f32r2" (high +
# residual pass), "f16res2" (f16 with patch+weight residual passes)
FWD_MODE = "f32r2"

_CACHE = {}


def _chunks():
    out = []
    h0 = 0
    while h0 < PATCH_ROWS:
        hb = min(HB, PATCH_ROWS - h0)
        out.append((h0, hb))
        h0 += hb
    return out


def _build(mode):
    key = ("nc", mode)
    if key in _CACHE:
        return _CACHE[key]
    import concourse.bacc as bacc
    import concourse.mybir as mybir
    import concourse.tile as tile
    from concourse.ap import AP

    f32 = mybir.dt.float32
    f32r = mybir.dt.float32r
    f16 = mybir.dt.float16
    Alu = mybir.AluOpType
    Act = mybir.ActivationFunctionType

    two_imgs = mode == "f16res2"
    pdt = f16 if mode == "f16res2" else f32r

    nc = bacc.Bacc(None, target_bir_lowering=False)
    if two_imgs:
        img = nc.dram_tensor("img", [BAND_ROWS, WPAD], f16, kind="ExternalInput")
        imgl = nc.dram_tensor("imgl", [BAND_ROWS, WPAD], f16, kind="ExternalInput")
    else:
        img = nc.dram_tensor("img", [BAND_ROWS, WPAD], f32r, kind="ExternalInput")
        imgl = None
    # forward lhsT chunks [K(patch idx), M(coeff)]; one or two passes
    n_wf = 2 if mode in ("f32r2", "f16res2") else 1
    wfs = []
    for i in range(n_wf):
        wfs.append((
            nc.dram_tensor(f"wfa{i}", [KA, PP], pdt, kind="ExternalInput"),
            nc.dram_tensor(f"wfb{i}", [KB, PP], pdt, kind="ExternalInput"),
        ))
    via = nc.dram_tensor("via", [MA, PP], f16, kind="ExternalInput")
    vib = nc.dram_tensor("vib", [MB, PP], f16, kind="ExternalInput")
    za = nc.dram_tensor("za", [MA, 1], f16, kind="ExternalInput")
    zb = nc.dram_tensor("zb", [MB, 1], f16, kind="ExternalInput")
    recm = nc.dram_tensor("recm", [PP, L], f16, kind="ExternalOutput")
    cnt = nc.dram_tensor("cnt", [1, L], f32, kind="ExternalOutput")

    COLS_MAX = HB * WO

    with tile.TileContext(nc) as tc:
        with (
            tc.tile_pool(name="consts", bufs=1) as consts,
            tc.tile_pool(name="pat", bufs=2) as pat,
            tc.tile_pool(name="work", bufs=3) as work,
            tc.tile_pool(name="cpool", bufs=2) as cpool,
            tc.tile_pool(name="psA", bufs=2, space="PSUM") as psA,
            tc.tile_pool(name="psB", bufs=2, space="PSUM") as psB,
        ):
            wts = []
            for i in range(n_wf):
                wA = consts.tile([KA, PP], pdt, tag=f"wA{i}")
                wB = consts.tile([KB, PP], pdt, tag=f"wB{i}")
                nc.sync.dma_start(wA[:], wfs[i][0][:, :])
                nc.sync.dma_start(wB[:], wfs[i][1][:, :])
                wts.append((wA, wB))
            vA = consts.tile([MA, PP], f16, tag="vA")
            vB = consts.tile([MB, PP], f16, tag="vB")
            zA = consts.tile([MA, 1], f16, tag="zA")
            zB = consts.tile([MB, 1], f16, tag="zB")
            nc.sync.dma_start(vA[:], via[:, :])
            nc.sync.dma_start(vB[:], vib[:, :])
            nc.sync.dma_start(zA[:], za[:, :])
            nc.sync.dma_start(zB[:], zb[:, :])

            for (h0, hb) in _chunks():
                cols = hb * WO
                base = h0 * WO

                # staged im2col: partition p=(di,dj), free (h, x)
                srcs = [img, imgl] if two_imgs else [img]
                ptiles = []
                for si, simg in enumerate(srcs):
                    pA = pat.tile([KA, COLS_MAX], pdt, tag=f"pA{si}")
                    pB = pat.tile([KB, COLS_MAX], pdt, tag=f"pB{si}")
                    for di in range(9):
                        src = AP(simg, (h0 + di) * WPAD, [[1, 13], [WPAD, hb], [1, WO]])
                        dst = pA[di * 13:(di + 1) * 13, 0:cols].rearrange(
                            "p (h x) -> p h x", h=hb)
                        nc.sync.dma_start(dst, src)
                    for di in range(9, 13):
                        src = AP(simg, (h0 + di) * WPAD, [[1, 13], [WPAD, hb], [1, WO]])
                        dst = pB[(di - 9) * 13:(di - 8) * 13, 0:cols].rearrange(
                            "p (h x) -> p h x", h=hb)
                        nc.sync.dma_start(dst, src)
                    ptiles.append((pA, pB))

                cntc = cpool.tile([1, COLS_MAX], f32, tag="cntc")

                c0 = 0
                while c0 < cols:
                    n = min(NT, cols - c0)
                    # forward accumulation group: per pass, per K chunk
                    t0 = psA.tile([MA, NT], f32, tag="t0")
                    t1 = psA.tile([MB, NT], f32, tag="t1")
                    groups = []
                    if mode == "f16res2":
                        # Wq@ph + Wq@pl + Wr@ph
                        groups = [(wts[0], ptiles[0]), (wts[0], ptiles[1]),
                                  (wts[1], ptiles[0])]
                    elif mode == "f32r2":
                        groups = [(wts[0], ptiles[0]), (wts[1], ptiles[0])]
                    else:
                        groups = [(wts[0], ptiles[0])]
                    ng = len(groups)
                    for gi, ((wA, wB), (pA, pB)) in enumerate(groups):
                        st = gi == 0
                        sp = gi == ng - 1
                        nc.tensor.matmul(t0[:, 0:n], wA[:, 0:MA],
                                         pA[:, c0:c0 + n], start=st, stop=False)
                        nc.tensor.matmul(t0[:, 0:n], wB[:, 0:MA],
                                         pB[:, c0:c0 + n], start=False, stop=sp)
                    for gi, ((wA, wB), (pA, pB)) in enumerate(groups):
                        st = gi == 0
                        sp = gi == ng - 1
                        nc.tensor.matmul(t1[:, 0:n], wA[:, MA:PP],
                                         pA[:, c0:c0 + n], start=st, stop=False)
                        nc.tensor.matmul(t1[:, 0:n], wB[:, MA:PP],
                                         pB[:, c0:c0 + n], start=False, stop=sp)

                    u0 = work.tile([MA, NT], f32, tag="u0")
                    u1 = work.tile([MB, NT], f32, tag="u1")
                    nc.scalar.activation(u0[:, 0:n], t0[:, 0:n], Act.Square)
                    nc.scalar.activation(u1[:, 0:n], t1[:, 0:n], Act.Square)
                    ib0 = work.tile([MA, NT], f16, tag="ib0")
                    ib1 = work.tile([MB, NT], f16, tag="ib1")
                    nc.vector.tensor_scalar(ib0[:, 0:n], u0[:, 0:n], 1.0, None,
                                            Alu.is_le)
                    nc.vector.tensor_scalar(ib1[:, 0:n], u1[:, 0:n], 1.0, None,
                                            Alu.is_le)

                    # count matmuls packed into spare partition 64 of the
                    # r1 PSUM bank (col tile_position 64)
                    r1c = psB.tile([65, NT], f32, tag="r1c")
                    cm1 = nc.tensor.matmul(r1c[64:65, 0:n], zA[:], ib0[:, 0:n],
                                           start=True, stop=False)
                    cm2 = nc.tensor.matmul(r1c[64:65, 0:n], zB[:], ib1[:, 0:n],
                                           start=False, stop=True)

                    z0 = work.tile([MA, NT], f16, tag="z0")
                    z1 = work.tile([MB, NT], f16, tag="z1")
                    nc.vector.tensor_mul(z0[:, 0:n], t0[:, 0:n], ib0[:, 0:n])
                    nc.vector.tensor_mul(z1[:, 0:n], t1[:, 0:n], ib1[:, 0:n])

                    r0 = psB.tile([KA, NT], f32, tag="r0")
                    nc.tensor.matmul(r0[:, 0:n], vA[:, 0:KA], z0[:, 0:n],
                                     start=True, stop=False)
                    nc.tensor.matmul(r0[:, 0:n], vB[:, 0:KA], z1[:, 0:n],
                                     start=False, stop=True)
                    im1 = nc.tensor.matmul(r1c[0:KB, 0:n], vA[:, KA:PP], z0[:, 0:n],
                                           start=True, stop=False)
                    nc.tensor.matmul(r1c[0:KB, 0:n], vB[:, KA:PP], z1[:, 0:n],
                                     start=False, stop=True)
                    # the count group and the r1 inverse group share a PSUM
                    # bank; their start=True bank-clears must not interleave
                    from concourse.tile_rust import add_dep_helper
                    add_dep_helper(im1.ins, cm2.ins, sync=False,
                                   reason="cnt group before r1 inverse group")

                    o0 = work.tile([KA, NT], f16, tag="o0")
                    o1 = work.tile([KB, NT], f16, tag="o1")
                    nc.scalar.copy(o0[:, 0:n], r0[:, 0:n])
                    nc.scalar.copy(o1[:, 0:n], r1c[0:KB, 0:n])
                    nc.vector.tensor_copy(cntc[0:1, c0:c0 + n], r1c[64:65, 0:n])

                    nc.sync.dma_start(recm[0:KA, base + c0: base + c0 + n],
                                      o0[:, 0:n])
                    nc.sync.dma_start(recm[KA:PP, base + c0: base + c0 + n],
                                      o1[:, 0:n])
                    c0 += n

                nc.sync.dma_start(cnt[0:1, base:base + cols], cntc[0:1, 0:cols])

    nc.compile()
    _CACHE[key] = nc
    return nc


LAST_EXEC_NS = None


def kernel(x, sigma_, Pm1, _trace=False):
    global LAST_EXEC_NS
    from concourse.bass_utils import run_bass_kernel_spmd

    x = np.asarray(x, np.float32)
    Pm1 = np.asarray(Pm1, np.float32)
    lam = 6.0 * float(np.asarray(sigma_).reshape(-1)[0])  # 3 * (2*sigma_)

    WfT = np.ascontiguousarray((Pm1 / lam).T.astype(np.float32))  # [p, k]
    Pinv64 = np.linalg.inv(Pm1.astype(np.float64))
    PinvT = np.ascontiguousarray((lam * Pinv64).T).astype(np.float32)  # [k, m]

    def f16(a):
        return np.ascontiguousarray(a.astype(np.float16))

    wf_passes = []
    if FWD_MODE == "f16res2":
        Wq = WfT.astype(np.float16)
        Wr = f16(WfT - Wq.astype(np.float32))
        wf_passes = [Wq, Wr]
    elif FWD_MODE == "f32r2":
        # fp32r keeps the "high" half of each fp32 weight; emulate the
        # split as bf16-high + residual so pass2 restores full precision
        import ml_dtypes
        Whi = WfT.astype(ml_dtypes.bfloat16).astype(np.float32)
        wf_passes = [np.ascontiguousarray(Whi),
                     np.ascontiguousarray(WfT - Whi)]
    else:
        wf_passes = [WfT]

    via = f16(PinvT[:MA])
    vib = f16(PinvT[MA:])
    za = np.ones((MA, 1), np.float16)
    za[0, 0] = 0.0
    zb = np.ones((MB, 1), np.float16)

    in_maps = []
    pads = []
    for nidx in range(N_IMG):
        imgf = 2.0 * x[nidx, 0] - 1.0
        pad = np.pad(imgf, P - 1, mode="reflect").astype(np.float32)
        pads.append(pad)
        for hbi in range(2):
            band = np.ascontiguousarray(
                pad[hbi * BAND_OUT: hbi * BAND_OUT + BAND_ROWS, :])
            m = {"via": via, "vib": vib, "za": za, "zb": zb}
            if FWD_MODE == "f16res2":
                bh = band.astype(np.float16)
                m["img"] = np.ascontiguousarray(bh)
                m["imgl"] = f16(band - bh.astype(np.float32))
            else:
                m["img"] = band
            for i, wp in enumerate(wf_passes):
                m[f"wfa{i}"] = np.ascontiguousarray(wp[:KA])
                m[f"wfb{i}"] = np.ascontiguousarray(wp[KA:])
            in_maps.append(m)

    nc = _build(FWD_MODE)
    import time as _time
    _t0 = _time.perf_counter()
    res = run_bass_kernel_spmd(nc, in_maps, core_ids=list(range(8)))
    _t1 = _time.perf_counter()
    LAST_EXEC_NS = res.exec_time_ns
    if LAST_EXEC_NS is None:
        LAST_EXEC_NS = int((_t1 - _t0) * 1e9)

    out = np.empty((N_IMG, 1, H, W), np.float32)
    for i in range(8):
        nidx, hbi = divmod(i, 2)
        rm = res.results[i]["recm"].astype(np.float32)
        cntv = res.results[i]["cnt"][0]
        w = (1.0 / (169.0 - cntv)).astype(np.float32)
        wg = w.reshape(PATCH_ROWS, WO)
        rr = (rm * w).reshape(P, P, PATCH_ROWS, WO)
        num = np.zeros((BAND_ROWS, WPAD), np.float32)
        div = np.zeros_like(num)
        for di in range(P):
            for dj in range(P):
                num[di:di + PATCH_ROWS, dj:dj + WO] += rr[di, dj]
                div[di:di + PATCH_ROWS, dj:dj + WO] += wg
        corr = num[P - 1:P - 1 + BAND_OUT, P - 1:P - 1 + W] \
            / div[P - 1:P - 1 + BAND_OUT, P - 1:P - 1 + W]
        out[nidx, 0, hbi * BAND_OUT:(hbi + 1) * BAND_OUT, :] = \
            x[nidx, 0, hbi * BAND_OUT:(hbi + 1) * BAND_OUT, :] - corr * 0.5
    return out


# revision 13
# speedup vs baseline: 1.3509x; 1.3509x over previous
"""DCT2net denoiser on 8 TRN2 NeuronCores.

Sharding: 8 cores = 4 images x 2 horizontal half-bands (data-parallel,
halo via overlapping patch bands -- no collectives).

Math: with forward weights Wf = Pm1/lam (threshold folded to +-1) and
z = t * 1{|t|<=1} (the sub-threshold coefficients), the hardshrink
reconstruction satisfies exactly

    rec = patches - lam*Pinv @ z

and since fold(w * shifted-copies-of-image) == image * fold(w), the
final output reduces to

    out = x - fold(w * recm)/fold(w) / 2,    recm := lam*Pinv @ z

so the device only computes t (f16 matmuls with hi/lo residual passes
over on-device im2col), z, cnt = #sub-threshold coeffs, and recm (f16
matmul).  Host does reflect-pad, the overlap-add fold and the final
division.

Device layout notes: patch free dim uses the full padded row pitch
(280) so the im2col DMA source is one contiguous run per (di,dj) --
columns x in [268,280) are garbage and are sliced off on the host.
The count matmul is packed into partition 64 of the r1 PSUM bank.
"""

import numpy as np

P = 13
PP = 169              # p*p
N_IMG, H, W = 4, 256, 256
BAND_OUT = 128        # output rows per core
PATCH_ROWS = BAND_OUT + P - 1        # 140
BAND_ROWS = BAND_OUT + 2 * (P - 1)   # 152
WPAD = W + 2 * (P - 1)               # 280
WO = W + P - 1        # 268 valid patch cols
LP = PATCH_ROWS * WPAD               # 39200 padded patch positions
KA, KB = 117, 52      # patch-row split: di groups 0..8 / 9..12
MA, MB = 128, 41      # coefficient split (k index)
NT = 512              # free-dim tile
HB = 21               # patch rows per staged chunk

# forward-transform mode: "f32r" (single high pass), "f32r2" (high +
# residual pass), "f16res2" (f16 with patch+weight residual passes)
FWD_MODE = "f16res2"

_CACHE = {}


def _chunks():
    out = []
    h0 = 0
    while h0 < PATCH_ROWS:
        hb = min(HB, PATCH_ROWS - h0)
        out.append((h0, hb))
        h0 += hb
    return out


def _build(mode, reps=1):
    key = ("nc", mode, reps)
    if key in _CACHE:
        return _CACHE[key]
    import concourse.bacc as bacc
    import concourse.mybir as mybir
    import concourse.tile as tile
    from concourse.ap import AP
    from concourse.tile_rust import add_dep_helper

    f32 = mybir.dt.float32
    f32r = mybir.dt.float32r
    f16 = mybir.dt.float16
    Alu = mybir.AluOpType
    Act = mybir.ActivationFunctionType

    two_imgs = mode == "f16res2"
    pdt = f16 if mode == "f16res2" else f32r

    nc = bacc.Bacc(None, target_bir_lowering=False)
    # one extra image row: the contiguous full-row im2col reads run up to
    # 12 elements past row BAND_ROWS-1 for the deepest di
    if two_imgs:
        img = nc.dram_tensor("img", [BAND_ROWS + 1, WPAD], f16, kind="ExternalInput")
        imgl = nc.dram_tensor("imgl", [BAND_ROWS + 1, WPAD], f16, kind="ExternalInput")
    else:
        img = nc.dram_tensor("img", [BAND_ROWS + 1, WPAD], f32r, kind="ExternalInput")
        imgl = None
    n_wf = 2 if mode in ("f32r2", "f16res2") else 1
    wfs = []
    for i in range(n_wf):
        wfs.append((
            nc.dram_tensor(f"wfa{i}", [KA, PP], pdt, kind="ExternalInput"),
            nc.dram_tensor(f"wfb{i}", [KB, PP], pdt, kind="ExternalInput"),
        ))
    via = nc.dram_tensor("via", [MA, PP], f16, kind="ExternalInput")
    vib = nc.dram_tensor("vib", [MB, PP], f16, kind="ExternalInput")
    za = nc.dram_tensor("za", [MA, 1], f16, kind="ExternalInput")
    zb = nc.dram_tensor("zb", [MB, 1], f16, kind="ExternalInput")
    recm = nc.dram_tensor("recm", [PP, LP], f16, kind="ExternalOutput")
    cnt = nc.dram_tensor("cnt", [1, LP], f32, kind="ExternalOutput")

    COLS_MAX = HB * WPAD

    with tile.TileContext(nc) as tc:
        with (
            tc.tile_pool(name="consts", bufs=1) as consts,
            tc.tile_pool(name="pat", bufs=2) as pat,
            tc.tile_pool(name="work", bufs=3) as work,
            tc.tile_pool(name="cpool", bufs=2) as cpool,
            tc.tile_pool(name="psA", bufs=2, space="PSUM") as psA,
            tc.tile_pool(name="psB", bufs=2, space="PSUM") as psB,
        ):
            wts = []
            for i in range(n_wf):
                wA = consts.tile([KA, PP], pdt, tag=f"wA{i}")
                wB = consts.tile([KB, PP], pdt, tag=f"wB{i}")
                nc.sync.dma_start(wA[:], wfs[i][0][:, :])
                nc.sync.dma_start(wB[:], wfs[i][1][:, :])
                wts.append((wA, wB))
            vA = consts.tile([MA, PP], f16, tag="vA")
            vB = consts.tile([MB, PP], f16, tag="vB")
            zA = consts.tile([MA, 1], f16, tag="zA")
            zB = consts.tile([MB, 1], f16, tag="zB")
            nc.sync.dma_start(vA[:], via[:, :])
            nc.sync.dma_start(vB[:], vib[:, :])
            nc.sync.dma_start(zA[:], za[:, :])
            nc.sync.dma_start(zB[:], zb[:, :])

            from contextlib import nullcontext
            rep_ctx = tc.For_i(0, reps, 1) if reps > 1 else nullcontext()
            with rep_ctx:
                _body(nc, tc, tile, AP, mybir, mode, two_imgs, pdt,
                      img, imgl, recm, cnt, wts, vA, vB, zA, zB,
                      pat, work, cpool, psA, psB, COLS_MAX)

    nc.compile()
    _CACHE[key] = nc
    return nc


def _body(nc, tc, tile, AP, mybir, mode, two_imgs, pdt,
          img, imgl, recm, cnt, wts, vA, vB, zA, zB,
          pat, work, cpool, psA, psB, COLS_MAX):
    from concourse.tile_rust import add_dep_helper
    f32 = mybir.dt.float32
    f16 = mybir.dt.float16
    Alu = mybir.AluOpType
    Act = mybir.ActivationFunctionType
    if True:
            for (h0, hb) in _chunks():
                cols = hb * WPAD
                base = h0 * WPAD

                # staged im2col, one 3D DMA per tile: partition (di,dj),
                # free = flat run of hb*280 elements starting at row
                # h0+di, column dj  (full-pitch rows -> contiguous src)
                srcs = [img, imgl] if two_imgs else [img]
                ptiles = []
                for si, simg in enumerate(srcs):
                    pA = pat.tile([KA, COLS_MAX], pdt, tag=f"pA{si}")
                    pB = pat.tile([KB, COLS_MAX], pdt, tag=f"pB{si}")
                    srcA = AP(simg, h0 * WPAD,
                              [[WPAD, 9], [1, 13], [1, cols]])
                    nc.sync.dma_start(pA[0:KA, 0:cols], srcA)
                    srcB = AP(simg, (h0 + 9) * WPAD,
                              [[WPAD, 4], [1, 13], [1, cols]])
                    nc.sync.dma_start(pB[0:KB, 0:cols], srcB)
                    ptiles.append((pA, pB))

                cntc = cpool.tile([1, COLS_MAX], f32, tag="cntc")

                c0 = 0
                while c0 < cols:
                    n = min(NT, cols - c0)
                    t0 = psA.tile([MA, NT], f32, tag="t0")
                    t1 = psA.tile([MB, NT], f32, tag="t1")
                    if mode == "f16res2":
                        # Wq@ph + Wq@pl + Wr@ph
                        groups = [(wts[0], ptiles[0]), (wts[0], ptiles[1]),
                                  (wts[1], ptiles[0])]
                    elif mode == "f32r2":
                        groups = [(wts[0], ptiles[0]), (wts[1], ptiles[0])]
                    else:
                        groups = [(wts[0], ptiles[0])]
                    ng = len(groups)
                    for gi, ((wA, wB), (pA, pB)) in enumerate(groups):
                        st = gi == 0
                        sp = gi == ng - 1
                        nc.tensor.matmul(t0[:, 0:n], wA[:, 0:MA],
                                         pA[:, c0:c0 + n], start=st, stop=False)
                        nc.tensor.matmul(t0[:, 0:n], wB[:, 0:MA],
                                         pB[:, c0:c0 + n], start=False, stop=sp)
                    for gi, ((wA, wB), (pA, pB)) in enumerate(groups):
                        st = gi == 0
                        sp = gi == ng - 1
                        nc.tensor.matmul(t1[:, 0:n], wA[:, MA:PP],
                                         pA[:, c0:c0 + n], start=st, stop=False)
                        nc.tensor.matmul(t1[:, 0:n], wB[:, MA:PP],
                                         pB[:, c0:c0 + n], start=False, stop=sp)

                    u0 = work.tile([MA, NT], f32, tag="u0")
                    u1 = work.tile([MB, NT], f32, tag="u1")
                    nc.scalar.activation(u0[:, 0:n], t0[:, 0:n], Act.Square)
                    nc.scalar.activation(u1[:, 0:n], t1[:, 0:n], Act.Square)
                    ib0 = work.tile([MA, NT], f16, tag="ib0")
                    ib1 = work.tile([MB, NT], f16, tag="ib1")
                    nc.vector.tensor_scalar(ib0[:, 0:n], u0[:, 0:n], 1.0, None,
                                            Alu.is_le)
                    nc.vector.tensor_scalar(ib1[:, 0:n], u1[:, 0:n], 1.0, None,
                                            Alu.is_le)

                    # count matmuls packed into spare partition 64 of the
                    # r1 PSUM bank (col tile_position 64)
                    r1c = psB.tile([65, NT], f32, tag="r1c")
                    cm1 = nc.tensor.matmul(r1c[64:65, 0:n], zA[:], ib0[:, 0:n],
                                           start=True, stop=False)
                    cm2 = nc.tensor.matmul(r1c[64:65, 0:n], zB[:], ib1[:, 0:n],
                                           start=False, stop=True)

                    z0 = work.tile([MA, NT], f16, tag="z0")
                    z1 = work.tile([MB, NT], f16, tag="z1")
                    nc.vector.tensor_mul(z0[:, 0:n], t0[:, 0:n], ib0[:, 0:n])
                    nc.vector.tensor_mul(z1[:, 0:n], t1[:, 0:n], ib1[:, 0:n])

                    r0 = psB.tile([KA, NT], f32, tag="r0")
                    nc.tensor.matmul(r0[:, 0:n], vA[:, 0:KA], z0[:, 0:n],
                                     start=True, stop=False)
                    nc.tensor.matmul(r0[:, 0:n], vB[:, 0:KA], z1[:, 0:n],
                                     start=False, stop=True)
                    im1 = nc.tensor.matmul(r1c[0:KB, 0:n], vA[:, KA:PP], z0[:, 0:n],
                                           start=True, stop=False)
                    nc.tensor.matmul(r1c[0:KB, 0:n], vB[:, KA:PP], z1[:, 0:n],
                                     start=False, stop=True)
                    # the count group and the r1 inverse group share a PSUM
                    # bank; their start=True bank-clears must not interleave
                    add_dep_helper(im1.ins, cm2.ins, sync=False,
                                   reason="cnt group before r1 inverse group")

                    o0 = work.tile([KA, NT], f16, tag="o0")
                    o1 = work.tile([KB, NT], f16, tag="o1")
                    nc.scalar.copy(o0[:, 0:n], r0[:, 0:n])
                    nc.scalar.copy(o1[:, 0:n], r1c[0:KB, 0:n])
                    nc.vector.tensor_copy(cntc[0:1, c0:c0 + n], r1c[64:65, 0:n])

                    nc.gpsimd.dma_start(recm[0:KA, base + c0: base + c0 + n],
                                        o0[:, 0:n])
                    nc.gpsimd.dma_start(recm[KA:PP, base + c0: base + c0 + n],
                                        o1[:, 0:n])
                    c0 += n

                nc.scalar.dma_start(cnt[0:1, base:base + cols], cntc[0:1, 0:cols])


LAST_EXEC_NS = None


def kernel(x, sigma_, Pm1, _trace=False):
    global LAST_EXEC_NS
    from concourse.bass_utils import run_bass_kernel_spmd

    x = np.asarray(x, np.float32)
    Pm1 = np.asarray(Pm1, np.float32)
    lam = 6.0 * float(np.asarray(sigma_).reshape(-1)[0])  # 3 * (2*sigma_)

    WfT = np.ascontiguousarray((Pm1 / lam).T.astype(np.float32))  # [p, k]
    Pinv64 = np.linalg.inv(Pm1.astype(np.float64))
    PinvT = np.ascontiguousarray((lam * Pinv64).T).astype(np.float32)  # [k, m]

    def f16(a):
        return np.ascontiguousarray(a.astype(np.float16))

    if FWD_MODE == "f16res2":
        Wq = WfT.astype(np.float16)
        Wr = f16(WfT - Wq.astype(np.float32))
        wf_passes = [Wq, Wr]
    elif FWD_MODE == "f32r2":
        import ml_dtypes
        Whi = WfT.astype(ml_dtypes.bfloat16).astype(np.float32)
        wf_passes = [np.ascontiguousarray(Whi),
                     np.ascontiguousarray(WfT - Whi)]
    else:
        wf_passes = [WfT]

    via = f16(PinvT[:MA])
    vib = f16(PinvT[MA:])
    za = np.ones((MA, 1), np.float16)
    za[0, 0] = 0.0
    zb = np.ones((MB, 1), np.float16)

    in_maps = []
    for nidx in range(N_IMG):
        imgf = 2.0 * x[nidx, 0] - 1.0
        pad = np.pad(imgf, P - 1, mode="reflect").astype(np.float32)
        for hbi in range(2):
            band = np.zeros((BAND_ROWS + 1, WPAD), np.float32)
            band[:BAND_ROWS] = pad[hbi * BAND_OUT: hbi * BAND_OUT + BAND_ROWS, :]
            m = {"via": via, "vib": vib, "za": za, "zb": zb}
            if FWD_MODE == "f16res2":
                bh = band.astype(np.float16)
                m["img"] = np.ascontiguousarray(bh)
                m["imgl"] = f16(band - bh.astype(np.float32))
            else:
                m["img"] = np.ascontiguousarray(band)
            for i, wp in enumerate(wf_passes):
                m[f"wfa{i}"] = np.ascontiguousarray(wp[:KA])
                m[f"wfb{i}"] = np.ascontiguousarray(wp[KA:])
            in_maps.append(m)

    nc = _build(FWD_MODE)
    import time as _time
    _t0 = _time.perf_counter()
    res = run_bass_kernel_spmd(nc, in_maps, core_ids=list(range(8)))
    _t1 = _time.perf_counter()
    LAST_EXEC_NS = res.exec_time_ns
    if LAST_EXEC_NS is None:
        LAST_EXEC_NS = int((_t1 - _t0) * 1e9)

    out = np.empty((N_IMG, 1, H, W), np.float32)
    for i in range(8):
        nidx, hbi = divmod(i, 2)
        # padded pitch-280 patch grid -> slice off the 12 garbage columns
        rm = res.results[i]["recm"].astype(np.float32)
        rm = rm.reshape(PP, PATCH_ROWS, WPAD)[:, :, :WO]
        cntv = res.results[i]["cnt"][0].reshape(PATCH_ROWS, WPAD)[:, :WO]
        w = (1.0 / (169.0 - cntv)).astype(np.float32)
        rr = rm.reshape(P, P, PATCH_ROWS, WO) * w
        num = np.zeros((BAND_ROWS, WPAD), np.float32)
        div = np.zeros_like(num)
        for di in range(P):
            for dj in range(P):
                num[di:di + PATCH_ROWS, dj:dj + WO] += rr[di, dj]
                div[di:di + PATCH_ROWS, dj:dj + WO] += w
        corr = num[P - 1:P - 1 + BAND_OUT, P - 1:P - 1 + W] \
            / div[P - 1:P - 1 + BAND_OUT, P - 1:P - 1 + W]
        out[nidx, 0, hbi * BAND_OUT:(hbi + 1) * BAND_OUT, :] = \
            x[nidx, 0, hbi * BAND_OUT:(hbi + 1) * BAND_OUT, :] - corr * 0.5
    return out
